# revision 27
# baseline (speedup 1.0000x reference)
"""Trainium2 Bass kernel for nn_Agent_14834817040470 (GPO/PPO-style loss_fn).

Data-parallel over B (256 envs) across 8 NeuronCores (32 envs/core).
Per core: normalize obs -> 3 MLPs (fp32r matmuls, feature-major activations,
bf16 h1+heads) -> sample-major heads -> GAE via tensor_tensor_scan (env-major,
reversed-time views) -> elementwise tail -> partial sums.  Host combines
partial sums and evaluates the adv-normalized policy-loss means (small numpy
finale over 1M elements).

Self-contained: hardcodes all shapes; only needs /opt/trn_rl_repo on sys.path.
"""
import math
import sys

import numpy as np

if "/opt/trn_rl_repo" not in sys.path:
    sys.path.insert(0, "/opt/trn_rl_repo")

import concourse.bacc as bacc
import concourse.bass as bass
import concourse.mybir as mybir
import concourse.tile as tile
from concourse import bass_utils

dt = mybir.dt
AF = mybir.ActivationFunctionType
ALU = mybir.AluOpType

GAMMA, LAM, EPSILON, EPS = 0.97, 0.95, 0.3, 0.2
ENT_COST, RSCALE, ALPHA, NUM_STEPS = 0.01, 1.0, 1.0, 1.0e6
HL2PI = 0.5 * math.log(2.0 * math.pi)
LOG2 = math.log(2.0)
T, B, OBS, ACTD, H = 1024, 256, 64, 8, 256
NCORES = 8
BL = B // NCORES           # 32 envs / core
S = T * BL                 # 32768 policy samples / core
SV = (T + 1) * BL          # 32800 value samples / core
C = S // 128               # 256 sample-major cols
NT = S // 512              # 64 tiles of 512 samples
F32, BF16, F32R = dt.float32, dt.bfloat16, dt.float32r

# Exp/Ln both live in natural_log_exp_and_others; the table chooser is greedy
# per-op and otherwise ping-pongs exp_and_others <-> natural_log (~1.3us per
# load). Strip Exp/Ln from every other set so the combined one is chosen.
_orig_gat = bacc.get_activation_tables


def _patched_gat(arch):
    tabs = {k: set(v) for k, v in _orig_gat(arch).items()}
    for name, funcs in tabs.items():
        if name != "natural_log_exp_and_others":
            funcs.discard(mybir.ActivationFunctionType.Exp)
            funcs.discard(mybir.ActivationFunctionType.Ln)
    return tabs


bacc.get_activation_tables = _patched_gat

_CACHE = {}


# ==========================================================================
# device program
# ==========================================================================
def build_program():
    nc = bacc.Bacc("TRN2", target_bir_lowering=False, debug=False,
                   num_devices=NCORES)

    def din(name, shape, dtype=F32):
        return nc.dram_tensor(name, list(shape), dtype, kind="ExternalInput").ap()

    t_in = dict(
        obsT=din("obsT", (OBS, SV), F32R),
        pobsT=din("pobsT", (OBS, SV), F32R),
        act_cm=din("act_cm", (128, C * ACTD)),
        noise_cm=din("noise_cm", (128, C * ACTD)),
        blog_cm=din("blog_cm", (128, C * 2 * ACTD)),
        rew_cm=din("rew_cm", (128, C)),
        done_cm=din("done_cm", (128, C)),
        trunc_cm=din("trunc_cm", (128, C)),
        w0pT=din("w0pT", (OBS, H), F32R),
        w1pT=din("w1pT", (H, H), F32R),
        w2pT_bf=din("w2pT_bf", (H, 16), BF16),
        w0vT=din("w0vT", (OBS, H), F32R),
        w1vT=din("w1vT", (H, H), F32R),
        w2vT_bf=din("w2vT_bf", (H, 16), BF16),
        b0p=din("b0p", (1, H)),
        b1p=din("b1p", (1, H)),
        b0v=din("b0v", (1, H)),
        b1v=din("b1v", (1, H)),
        head_bias=din("head_bias", (128, 48)),
        nrm=din("nrm", (1, 2 * OBS)),
    )
    out_scal = nc.dram_tensor("out_scal", [128, 8], F32, kind="ExternalOutput").ap()
    out_arrs = nc.dram_tensor("out_arrs", [4, 128, C], F32, kind="ExternalOutput").ap()

    with tile.TileContext(nc) as tc:
        _kernel_body(nc, tc, t_in, out_scal, out_arrs)
    nc.compile()
    return nc


def _kernel_body(nc, tc, t_in, out_scal, out_arrs):
    ts = bass.ts

    def r(ap):  # fp32 -> fp32r view: 1 cyc/row matmuls at N>=256
        return ap.bitcast(F32R)

    with tc.tile_pool(name="const", bufs=1) as cpool, \
         tc.tile_pool(name="resident", bufs=1) as res:
        # ---- constants -------------------------------------------------
        w0p_sb = cpool.tile([128, H], F32R, name="w0p_sb")
        nc.sync.dma_start(w0p_sb[0:OBS, :], t_in["w0pT"])
        nc.sync.dma_start(w0p_sb[OBS:128, :], t_in["w0pT"])
        w0v_sb = cpool.tile([128, H], F32R, name="w0v_sb")
        nc.sync.dma_start(w0v_sb[0:OBS, :], t_in["w0vT"])
        nc.sync.dma_start(w0v_sb[OBS:128, :], t_in["w0vT"])
        w1p_sb = cpool.tile([128, 2 * H], F32R, name="w1p_sb")  # k-chunk halves
        nc.sync.dma_start(w1p_sb[:, 0:H], t_in["w1pT"][0:128, :])
        nc.sync.dma_start(w1p_sb[:, H:2 * H], t_in["w1pT"][128:256, :])
        w1v_sb = cpool.tile([128, 2 * H], F32R, name="w1v_sb")
        nc.sync.dma_start(w1v_sb[:, 0:H], t_in["w1vT"][0:128, :])
        nc.sync.dma_start(w1v_sb[:, H:2 * H], t_in["w1vT"][128:256, :])
        w2p_sb = cpool.tile([128, 32], BF16, name="w2p_sb")
        nc.sync.dma_start(w2p_sb[:, 0:16], t_in["w2pT_bf"][0:128, :])
        nc.sync.dma_start(w2p_sb[:, 16:32], t_in["w2pT_bf"][128:256, :])
        w2v_sb = cpool.tile([128, 32], BF16, name="w2v_sb")
        nc.sync.dma_start(w2v_sb[:, 0:16], t_in["w2vT_bf"][0:128, :])
        nc.sync.dma_start(w2v_sb[:, 16:32], t_in["w2vT_bf"][128:256, :])

        def bias2(name):
            b = cpool.tile([128, 2], F32, name=f"{name}_sb")
            nc.sync.dma_start(b, t_in[name].rearrange("o (m p) -> (o p) m", m=2))
            return b

        bias0p, bias1p = bias2("b0p"), bias2("b1p")
        bias0v, bias1v = bias2("b0v"), bias2("b1v")
        hbias = cpool.tile([128, 48], F32, name="hbias_sb")
        nc.sync.dma_start(hbias, t_in["head_bias"])
        nrm_sb = cpool.tile([128, 2], F32, name="nrm_sb")  # col0 istd col1 nbias
        nrm_r = t_in["nrm"].rearrange("o (k p) -> (o p) k", k=2)
        nc.sync.dma_start(nrm_sb[0:64, :], nrm_r)
        nc.sync.dma_start(nrm_sb[64:128, :], nrm_r)

        # ---- resident sample-major head outputs ------------------------
        glog = res.tile([128, C, 16], F32, name="glog")
        llog = res.tile([128, C, 16], F32, name="llog")
        vals = res.tile([128, C + 4], F32, name="vals")

        # ---- MLP over 1024-sample tile-pairs (silu at FD=1024) ----------
        NP = NT // 2   # 32 pairs + one final 512 tile (value bootstrap pad)
        last_silus = []
        with tc.tile_pool(name="mlp_sb", bufs=2) as sbp, \
             tc.tile_pool(name="h0ps_pool", bufs=2, space="PSUM") as psp0, \
             tc.tile_pool(name="h1ps_pool", bufs=2, space="PSUM") as psp1, \
             tc.tile_pool(name="headps_pool", bufs=2, space="PSUM") as psph:
            for i in range(NP + 1):
                last = i == NP
                n = 512 if last else 1024
                # last tile: window over the final 512 value-stream samples;
                # only the trailing 32 (the bootstrap row) are consumed.
                s0 = SV - 512 if last else 1024 * i
                xn = sbp.tile([128, n], F32R, tag="xn", name=f"xn{i}",
                              padded_shape=[128, 1024])
                nc.sync.dma_start(xn[0:64, :], t_in["obsT"][:, s0:s0 + n])
                if not last:
                    nc.sync.dma_start(xn[64:128, :], t_in["pobsT"][:, s0:s0 + n])
                else:
                    nc.sync.dma_start(xn[64:128, :], t_in["pobsT"][:, s0:s0 + n])
                nc.vector.tensor_scalar(xn, xn, nrm_sb[:, 0:1], nrm_sb[:, 1:2],
                                        ALU.mult, ALU.add)
                nc.vector.tensor_scalar(xn, xn, 5.0, -5.0, ALU.min, ALU.max)

                streams = []
                for sname in (("v",) if last else ("g", "l", "v")):
                    if sname == "g":
                        w0s, w1s, bi0, bi1, xp, w2s, colb = \
                            w0p_sb, w1p_sb, bias0p, bias1p, 0, w2p_sb, 0
                    elif sname == "l":
                        w0s, w1s, bi0, bi1, xp, w2s, colb = \
                            w0p_sb, w1p_sb, bias0p, bias1p, 64, w2p_sb, 16
                    else:
                        w0s, w1s, bi0, bi1, xp, w2s, colb = \
                            w0v_sb, w1v_sb, bias0v, bias1v, 0, w2v_sb, 32
                    h0_sb = sbp.tile([128, 2 * n], F32R, tag=f"h0{sname}",
                                     name=f"h0{sname}{i}",
                                     padded_shape=[128, 2048])
                    for m in range(2):
                        h0_ps = psp0.tile([128, n], F32, tag="h0ps",
                                          name=f"h0ps{sname}{i}{m}",
                                          padded_shape=[128, 1024])
                        for half in range(n // 512):
                            nc.tensor.matmul(h0_ps[:, ts(half, 512)],
                                             w0s[xp:xp + 64, ts(m, 128)],
                                             xn[xp:xp + 64, ts(half, 512)])
                        si = nc.scalar.activation(h0_sb[:, ts(m, n)], h0_ps,
                                                  AF.Silu, bias=bi0[:, m:m + 1])
                    h1_sb = sbp.tile([128, 2 * n], BF16, tag=f"h1{sname}",
                                     name=f"h1{sname}{i}",
                                     padded_shape=[128, 2048])
                    for m in range(2):
                        for half in range(n // 512):
                            h1_ps = psp1.tile([128, 512], F32, tag="h1ps",
                                              name=f"h1ps{sname}{i}{m}{half}")
                            for k in range(2):
                                nc.tensor.matmul(h1_ps,
                                                 w1s[:, 256 * k + 128 * m:
                                                       256 * k + 128 * m + 128],
                                                 h0_sb[:, k * n + 512 * half:
                                                       k * n + 512 * half + 512],
                                                 start=(k == 0), stop=(k == 1))
                            si = nc.scalar.activation(
                                h1_sb[:, m * n + 512 * half: m * n + 512 * half + 512],
                                h1_ps, AF.Silu, bias=bi1[:, m:m + 1])
                            last_silus.append(si)
                    streams.append((sname, h1_sb, w2s, colb))

                # heads: lhsT = h1 sample-slice, rhs = w2 k-chunk
                hps = psph.tile([128, 8, 48], F32, tag="headps", name=f"hps{i}")
                for cc in ([3] if last else range(8)):
                    for sname, h1_sb, w2s, colb in streams:
                        for k in range(2):
                            nc.tensor.matmul(
                                hps[:, cc, colb:colb + 16],
                                h1_sb[:, k * n + 128 * cc: k * n + 128 * cc + 128],
                                w2s[:, ts(k, 16)],
                                start=(k == 0), stop=(k == 1))
                if not last:
                    hb16 = hbias[:, 0:16].unsqueeze(1).broadcast_to([128, 8, 16])
                    nc.vector.tensor_tensor(glog[:, 8 * i:8 * i + 8, :],
                                            hps[:, :, 0:16], hb16, ALU.add)
                    hb16l = hbias[:, 16:32].unsqueeze(1).broadcast_to([128, 8, 16])
                    nc.vector.tensor_tensor(llog[:, 8 * i:8 * i + 8, :],
                                            hps[:, :, 16:32], hb16l, ALU.add)
                    hb1 = hbias[:, 32:33].broadcast_to([128, 8])
                    nc.vector.tensor_tensor(vals[:, 8 * i:8 * i + 8],
                                            hps[:, :, 32], hb1, ALU.add)
                else:
                    # bootstrap samples 32768..32800 = window chunk 3, parts 96:128
                    nc.vector.tensor_tensor(vals[0:BL, C:C + 1],
                                            hps[96:128, 3, 32:33],
                                            hbias[0:BL, 32:33], ALU.add)
        nc._last_mlp_silu = last_silus[-1] if last_silus else None

        # ---- GAE + tail -------------------------------------------------
        import os
        if os.environ.get("K_STAGE", "full") == "mlp":
            with tc.tile_pool(name="dbg", bufs=1) as dbg:
                fin = dbg.tile([128, 8], F32, name="fin")
                nc.vector.tensor_copy(fin, vals[:, 0:8])
                nc.sync.dma_start(out_scal, fin[0:1, :])
                nc.sync.dma_start(out_arrs[0], vals[:, 0:C])
                nc.sync.dma_start(out_arrs[1], glog.rearrange("p c d -> p (c d)")[:, 0:C])
        else:
            _gae_and_tail(nc, tc, t_in, glog, llog, vals, out_scal, out_arrs)


def _gae_and_tail(nc, tc, t_in, glog, llog, vals, out_scal, out_arrs):
    from concourse.bass import _add_dep_helper
    TT = nc.vector.tensor_tensor
    TS = nc.vector.tensor_scalar
    STT = nc.vector.scalar_tensor_tensor
    GTT = nc.gpsimd.tensor_tensor
    GTS = nc.gpsimd.tensor_scalar
    GCP = nc.gpsimd.tensor_copy
    _ls = getattr(nc, "_last_mlp_silu", None)

    def ACTV(*a, **k):
        # order all tail ACT ops after the MLP silus so the activation
        # table set switches only once (silu set -> ln/exp set)
        inst = nc.scalar.activation(*a, **k)
        if _ls is not None:
            _add_dep_helper(inst.ins, _ls.ins, sync=False,
                            reason="act-table phase ordering")
        return inst

    CH = C // 4  # process dim-8 tail in four sample-quarters (overlap + SBUF)

    with tc.tile_pool(name="tail", bufs=1) as tp:
        def t8(name):
            return tp.tile([128, CH, ACTD], F32, name=name)

        # ---------- GAE (env-major, forward layout; scan on reversed views)
        rew_s = tp.tile([128, C], F32, name="rew_s")
        done_s = tp.tile([128, C], F32, name="done_s")
        trunc_s = tp.tile([128, C], F32, name="trunc_s")
        nc.sync.dma_start(rew_s, t_in["rew_cm"])
        nc.sync.dma_start(done_s, t_in["done_cm"])
        nc.sync.dma_start(trunc_s, t_in["trunc_cm"])

        def to_em(dst_em, src_cm, ncols=C):
            d3 = dst_em.rearrange("b (c q) -> b c q", q=4)
            for q in range(4):
                GCP(d3[:, :, q], src_cm[32 * q:32 * q + 32, :])

        rew_e = tp.tile([BL, T], F32, name="rew_e")
        tm_e = tp.tile([BL, T], F32, name="tm_e")
        z_e = tp.tile([BL, T], F32, name="z_e")
        vals_e = tp.tile([BL, T + 1], F32, name="vals_e")
        to_em(rew_e, rew_s)
        to_em(tm_e, trunc_s)
        to_em(z_e, done_s)
        ve3 = vals_e[:, 0:T].rearrange("b (c q) -> b c q", q=4)
        for q in range(4):
            GCP(ve3[:, :, q], vals[32 * q:32 * q + 32, 0:C])
        GCP(vals_e[:, T:T + 1], vals[0:BL, C:C + 1])

        # tm = 1-trunc ; te = done*tm ; z = gamma*(1-te)
        TS(tm_e, tm_e, -1.0, 1.0, ALU.mult, ALU.add)
        GTT(z_e, z_e, tm_e, ALU.mult)                      # te
        TS(z_e, z_e, -GAMMA, GAMMA, ALU.mult, ALU.add)    # gamma*(1-te)
        # deltas = (rew*RS + z*v[t+1] - v[t]) * tm
        d_e = tp.tile([BL, T], F32, name="d_e")
        GTT(d_e, z_e, vals_e[:, 1:T + 1], ALU.mult)
        GTT(d_e, d_e, rew_e, ALU.add)                      # RSCALE == 1
        GTT(d_e, d_e, vals_e[:, 0:T], ALU.subtract)
        GTT(d_e, d_e, tm_e, ALU.mult)
        coef2_e = tp.tile([BL, T], F32, name="coef2_e")
        GTT(coef2_e, z_e, tm_e, ALU.mult)                  # gamma*(1-te)*tm
        c_e = tp.tile([BL, T], F32, name="c_e")
        TS(c_e, coef2_e, LAM, None, ALU.mult)
        # scan (reverse time): state = c*state + d
        vmx_e = tp.tile([BL, T + 1], F32, name="vmx_e")   # col T == 0
        nc.vector.memset(vmx_e[:, T:T + 1], 0.0)
        nc.vector.tensor_tensor_scan(vmx_e[:, 0:T][:, ::-1],
                                     c_e[:, ::-1], d_e[:, ::-1],
                                     0.0, ALU.mult, ALU.add)
        # adv = d + coef2 * vmx[t+1]
        adv_e = tp.tile([BL, T], F32, name="adv_e")
        GTT(adv_e, coef2_e, vmx_e[:, 1:T + 1], ALU.mult)
        GTT(adv_e, adv_e, d_e, ALU.add)

        import os
        if os.environ.get("K_STAGE", "full") == "gae":
            adv_s = tp.tile([128, C], F32, name="adv_s")
            a3 = adv_e.rearrange("b (c q) -> b c q", q=4)
            for q in range(4):
                nc.vector.tensor_copy(adv_s[32 * q:32 * q + 32, :], a3[:, :, q])
            nc.sync.dma_start(out_arrs[3], adv_s)
            fin_sb0 = tp.tile([1, 8], F32, name="fin_sb0")
            nc.vector.tensor_copy(fin_sb0, adv_s[0:1, 0:8])
            nc.sync.dma_start(out_scal, fin_sb0)
            return

        # ---------- tail: distributions & log-probs (sample-major) -------
        c001 = tp.tile([128, 1], F32, name="c001")
        nc.vector.memset(c001, 0.001)
        cone = tp.tile([128, 1], F32, name="cone")
        nc.vector.memset(cone, 1.0)

        b_lp = tp.tile([128, C], F32, name="b_lp")
        g_lp = tp.tile([128, C], F32, name="g_lp")
        l_lp = tp.tile([128, C], F32, name="l_lp")
        kl = tp.tile([128, C], F32, name="kl")
        ent = tp.tile([128, C], F32, name="ent")
        KLP = 8.0 * (HL2PI + 2.0 * LOG2)
        a_cm3 = t_in["act_cm"].rearrange("p (c d) -> p c d", d=ACTD)
        n_cm3 = t_in["noise_cm"].rearrange("p (c d) -> p c d", d=ACTD)
        b_cm3 = t_in["blog_cm"].rearrange("p (c d) -> p c d", d=16)

        for h in range(4):
            def t8(nm, h=h):
                return tp.tile([128, CH, ACTD], F32, tag=nm,
                               name=f"{nm}{h}", bufs=2)

            act_sb = t8("act_sb")
            noise_sb = t8("noise_sb")
            blog_sb = tp.tile([128, CH, 16], F32, tag="blog_sb",
                              name=f"blog_sb{h}", bufs=2)
            sp_b, sp_g, sp_l = t8("sp_b"), t8("sp_g"), t8("sp_l")
            spa, w_a = t8("spa"), t8("w_a")
            edist, spe = t8("edist"), t8("spe")
            logs_b, logs_g, logs_l = t8("logs_b"), t8("logs_g"), t8("logs_l")
            inv2_b, inv2_g, inv2_l = t8("inv2_b"), t8("inv2_g"), t8("inv2_l")
            u, t4 = t8("u"), t8("t4")
            kscr, escr = t8("kscr"), t8("escr")
            dlogt, vr = t8("dlogt"), t8("vr")
            cs = slice(h * CH, (h + 1) * CH)
            glog_h, llog_h = glog[:, cs, :], llog[:, cs, :]
            nc.sync.dma_start(act_sb, a_cm3[:, cs, :])
            nc.sync.dma_start(noise_sb, n_cm3[:, cs, :])
            nc.sync.dma_start(blog_sb, b_cm3[:, cs, :])

            # softplus(scale*x) = relu(scale*x) + ln(1 + exp(-|scale*x|))
            def softplus_into(sp, x, scale=1.0):
                ACTV(u, x, AF.Abs)
                ACTV(u, u, AF.Exp, scale=-abs(scale))
                ACTV(t4, u, AF.Ln, bias=cone[:, 0:1])
                ACTV(sp, x, AF.Relu, scale=scale)
                TT(sp, sp, t4, ALU.add)

            def softplus_fast(sp, x, scale):
                # softplus(scale*x) = ln(1 + exp(scale*x)); |scale*x| small
                ACTV(u, x, AF.Exp, scale=scale)
                ACTV(sp, u, AF.Ln, bias=cone[:, 0:1])

            softplus_into(sp_b, blog_sb[:, :, 8:16])
            softplus_into(sp_g, glog_h[:, :, 8:16])
            softplus_into(sp_l, llog_h[:, :, 8:16])
            softplus_fast(spa, act_sb, -2.0)
            # edist = bloc + (sp_b + 0.001)*noise
            TS(edist, sp_b, 0.001, None, ALU.add)
            GTT(edist, edist, noise_sb, ALU.mult)
            GTT(edist, edist, blog_sb[:, :, 0:8], ALU.add)
            softplus_fast(spe, edist, -2.0)
            # ln phase (fused +0.001)
            ACTV(logs_b, sp_b, AF.Ln, bias=c001[:, 0:1])
            ACTV(logs_g, sp_g, AF.Ln, bias=c001[:, 0:1])
            ACTV(logs_l, sp_l, AF.Ln, bias=c001[:, 0:1])
            # exp phase: inv2 = exp(-2*logs)
            ACTV(inv2_b, logs_b, AF.Exp, scale=-2.0)
            ACTV(inv2_g, logs_g, AF.Exp, scale=-2.0)
            ACTV(inv2_l, logs_l, AF.Exp, scale=-2.0)
            # vr = exp(2*(logs_g - logs_l))
            TT(dlogt, logs_g, logs_l, ALU.subtract)
            ACTV(vr, dlogt, AF.Exp, scale=2.0)

            # w_a = act + softplus(-2 act)
            GTT(w_a, act_sb, spa, ALU.add)

            def log_prob(out, loc, logs, inv2):
                TT(u, act_sb, loc, ALU.subtract)
                TT(u, u, u, ALU.mult)
                TT(u, u, inv2, ALU.mult)
                STT(t4, u, 0.5, logs, ALU.mult, ALU.add)    # 0.5 u + logs
                STT(t4, w_a, -2.0, t4, ALU.mult, ALU.add)   # -2 w_a + ...
                nc.vector.tensor_reduce(out, t4, mybir.AxisListType.X, ALU.add)
                TS(out, out, KLP, -1.0, ALU.add, ALU.mult)  # -(sum + 8K)

            log_prob(b_lp[:, cs], blog_sb[:, :, 0:8], logs_b, inv2_b)
            log_prob(g_lp[:, cs], glog_h[:, :, 0:8], logs_g, inv2_g)
            log_prob(l_lp[:, cs], llog_h[:, :, 0:8], logs_l, inv2_l)

            # kl per sample: (sum_d 0.5*(vr + t1 - 1 - 2 dlog)) / 8
            GTT(kscr, glog_h[:, :, 0:8], llog_h[:, :, 0:8], ALU.subtract)
            GTT(kscr, kscr, kscr, ALU.mult)
            GTT(kscr, kscr, inv2_l, ALU.mult)               # t1
            GTT(kscr, kscr, vr, ALU.add)
            STT(kscr, dlogt, -2.0, kscr, ALU.mult, ALU.add)
            nc.vector.tensor_reduce(kl[:, cs], kscr, mybir.AxisListType.X, ALU.add)
            TS(kl[:, cs], kl[:, cs], -8.0, 1.0 / 16.0, ALU.add, ALU.mult)

            # entropy per sample
            GTT(escr, edist, spe, ALU.add)
            STT(escr, escr, -2.0, logs_b, ALU.mult, ALU.add)
            nc.vector.tensor_reduce(ent[:, cs], escr, mybir.AxisListType.X, ALU.add)
            TS(ent[:, cs], ent[:, cs], 8.0 * (0.5 + HL2PI + 2.0 * LOG2),
               None, ALU.add)

        # masks & rhos
        diff = tp.tile([128, C], F32, name="diff")
        m1 = tp.tile([128, C], F32, name="m1")
        klm = tp.tile([128, C], F32, name="klm")
        TT(diff, g_lp, l_lp, ALU.subtract)
        TS(m1, diff, math.log(1.0 + EPS), None, ALU.is_gt)
        TS(klm, diff, math.log(1.0 - EPS), None, ALU.is_lt)
        TT(m1, m1, klm, ALU.add)
        TT(klm, kl, m1, ALU.mult)

        rho = tp.tile([128, C], F32, name="rho")
        rho_clip = tp.tile([128, C], F32, name="rho_clip")
        rho2 = tp.tile([128, C], F32, name="rho2")
        lb = tp.tile([128, C], F32, name="lb")
        TT(rho, g_lp, b_lp, ALU.subtract)
        ACTV(rho, rho, AF.Exp)
        TT(lb, l_lp, b_lp, ALU.subtract)
        TS(rho_clip, diff, math.log(1.0 + EPS), math.log(1.0 - EPS),
           ALU.min, ALU.max)
        TT(rho_clip, rho_clip, lb, ALU.add)
        ACTV(rho_clip, rho_clip, AF.Exp)
        TS(rho_clip, rho_clip, 1.0 + EPSILON, 1.0 - EPSILON, ALU.min, ALU.max)
        TS(rho2, lb, 10.0, None, ALU.min)
        ACTV(rho2, rho2, AF.Exp)

        # adv back to sample-major
        adv_s = tp.tile([128, C], F32, name="adv_s")
        a3 = adv_e.rearrange("b (c q) -> b c q", q=4)
        for q in range(4):
            nc.vector.tensor_copy(adv_s[32 * q:32 * q + 32, :], a3[:, :, q])

        # ---------- partial sums -----------------------------------------
        sums = tp.tile([128, 8], F32, name="sums")
        nc.vector.memset(sums, 0.0)
        nc.vector.tensor_reduce(sums[:, 0:1], kl, mybir.AxisListType.X, ALU.add)
        nc.vector.tensor_reduce(sums[:, 1:2], klm, mybir.AxisListType.X, ALU.add)
        nc.vector.tensor_reduce(sums[:, 2:3], ent, mybir.AxisListType.X, ALU.add)
        scr_e = tp.tile([BL, T], F32, name="scr_e")
        GTT(scr_e, vmx_e[:, 0:T], vmx_e[:, 0:T], ALU.mult)
        nc.vector.tensor_reduce(sums[0:BL, 3:4], scr_e, mybir.AxisListType.X,
                                ALU.add)
        nc.vector.tensor_reduce(sums[0:BL, 4:5], adv_e, mybir.AxisListType.X,
                                ALU.add)
        GTT(scr_e, adv_e, adv_e, ALU.mult)
        nc.vector.tensor_reduce(sums[0:BL, 5:6], scr_e, mybir.AxisListType.X,
                                ALU.add)
        # cross-partition summation happens on the host (out_scal is [128, 8])
        nc.sync.dma_start(out_scal, sums)

        nc.sync.dma_start(out_arrs[0], rho)
        nc.sync.dma_start(out_arrs[1], rho_clip)
        nc.sync.dma_start(out_arrs[2], rho2)
        nc.sync.dma_start(out_arrs[3], adv_s)


# ==========================================================================
# host side
# ==========================================================================
def _f32(x):
    return np.asarray(x, np.float32)


def host_prep(inputs):
    import ml_dtypes
    rm, rv = _f32(inputs["running_mean"]), _f32(inputs["running_variance"])
    var = np.clip(rv / (NUM_STEPS + 1.0), 1e-6, 1e6)
    istd = _f32(1.0 / np.sqrt(var))
    nbias = _f32(-rm * istd)

    def wt(w):
        return _f32(np.ascontiguousarray(_f32(w).T))

    w2p = np.ascontiguousarray(_f32(inputs["pw2"]).T).astype(ml_dtypes.bfloat16)
    w2v = np.zeros((H, 16), np.float32)
    w2v[:, 0] = _f32(inputs["vw2"])[0, :]
    w2v = w2v.astype(ml_dtypes.bfloat16)
    hb = np.zeros((48,), np.float32)
    hb[0:16] = _f32(inputs["pb2"])
    hb[16:32] = _f32(inputs["pb2"])
    hb[32] = _f32(inputs["vb2"])[0]
    shared = dict(
        w0pT=wt(inputs["pw0"]), w1pT=wt(inputs["pw1"]), w2pT_bf=w2p,
        w0vT=wt(inputs["vw0"]), w1vT=wt(inputs["vw1"]), w2vT_bf=w2v,
        b0p=_f32(inputs["pb0"]).reshape(1, H), b1p=_f32(inputs["pb1"]).reshape(1, H),
        b0v=_f32(inputs["vb0"]).reshape(1, H), b1v=_f32(inputs["vb1"]).reshape(1, H),
        head_bias=_f32(np.broadcast_to(hb, (128, 48))).copy(),
        nrm=np.concatenate([istd, nbias]).reshape(1, 2 * OBS),
    )

    def cm(x, d):  # [S, d] -> [128, C*d] col-major sample blocks
        return _f32(np.ascontiguousarray(
            x.reshape(C, 128, d).transpose(1, 0, 2).reshape(128, C * d)))

    cores = []
    for i in range(NCORES):
        sl = slice(i * BL, (i + 1) * BL)
        core = dict(shared)
        core["obsT"] = _f32(np.ascontiguousarray(
            _f32(inputs["observation"][:, sl, :]).reshape(SV, OBS).T))
        core["pobsT"] = _f32(np.ascontiguousarray(
            _f32(inputs["pobservation"][:, sl, :]).reshape(SV, OBS).T))
        core["act_cm"] = cm(_f32(inputs["action"][:, sl, :]).reshape(S, ACTD), ACTD)
        core["noise_cm"] = cm(_f32(inputs["noise"][:, sl, :]).reshape(S, ACTD), ACTD)
        core["blog_cm"] = cm(_f32(inputs["logits"][:, sl, :]).reshape(S, 16), 16)
        core["rew_cm"] = cm(_f32(inputs["reward"][:, sl]).reshape(S, 1), 1)
        core["done_cm"] = cm(_f32(inputs["done"][:, sl]).reshape(S, 1), 1)
        core["trunc_cm"] = cm(_f32(inputs["truncation"][:, sl]).reshape(S, 1), 1)
        cores.append(core)
    return cores


def host_finale(scals, arrs):
    N = float(T * B)
    tot = np.sum(np.stack(scals), axis=0)          # [8] float64
    sum_kl, sum_klm, sum_ent, sum_verr2, sum_adv, sum_adv2 = tot[:6]
    mean_adv = sum_adv / N
    var_adv = (sum_adv2 - N * mean_adv ** 2) / (N - 1.0)
    std = math.sqrt(max(var_adv, 0.0)) + 1e-5
    gsum = lsum = 0.0
    for a in arrs:
        rho, rho_clip, rho2, adv = (a[j].astype(np.float64) for j in range(4))
        advn = (adv - mean_adv) / std
        gsum += np.minimum(rho * advn, rho_clip * advn).sum()
        r2c = np.clip(rho2, 1.0 - EPSILON, 1.0 + EPSILON)
        lsum += np.minimum(rho2 * advn, r2c * advn).sum()
    guider_policy_loss = -gsum / N
    learner_policy_loss = -lsum / N
    kl_learner = sum_kl / N
    kl_guider = sum_klm / N
    v_loss = sum_verr2 / N * 0.25
    entropy_loss = -ENT_COST * (sum_ent / N)
    learner_loss = kl_learner + learner_policy_loss * ALPHA
    guider_loss = kl_guider + guider_policy_loss
    total = learner_loss + guider_loss + v_loss + entropy_loss
    return np.array([total, guider_loss, learner_loss, kl_learner], np.float32)


def kernel(**inputs):
    if "nc" not in _CACHE:
        _CACHE["nc"] = build_program()
    nc = _CACHE["nc"]
    in_maps = host_prep(inputs)
    res = bass_utils.run_bass_kernel_spmd(nc, in_maps, core_ids=list(range(NCORES)))
    scals = [r["out_scal"].astype(np.float64).sum(axis=0) for r in res.results]
    arrs = [r["out_arrs"] for r in res.results]
    return host_finale(scals, arrs)


if __name__ == "__main__":
    rng = np.random.default_rng(0)
    print("building program...")
    ncp = build_program()
    print("instructions:", sum(len(f.instructions) for f in ncp.m.functions))
